# revision 8
# baseline (speedup 1.0000x reference)
"""Trainium2 Bass kernel: multi-head attention (B=2, S=2048, H=768, 12 heads x 64).

Sharding: 24 (batch, head) pairs over 8 cores -> 3 heads of one batch per core
(pure data/head parallel, no collectives; outputs gathered host-side).

Per-core pipeline (matmul operands bf16, accumulation fp32):
  A. DMA hs[b] (2048x768), joint W=[Wq|Wk|Wv] local columns (768x576), eye128.
  B. PE-transpose hs -> hsT (768 on partitions).
  C1. K,V projections first (scores need all kv columns of K^T): K^T heads 0,1
      share a tile on opposite partition halves; head 2 replicated on both
      halves. V via PE transpose of V^T -> v1 [128, h, t, 64].
  D. Per q-chunk (512): Q^T chunk projection, then per head: S^T = K^T.T @ Q^T
      (heads 0/1 interleaved so adjacent matmuls use disjoint PE row-groups and
      run concurrently; head 2 alternates its replicated halves), exp on
      ScalarE (PSUM->SBUF, scale=1/8; scores ~ N(0,1) so no max subtraction),
      denominators d_h = ones.T @ E^T as three concurrent M=1 column-tiles,
      ctx^T for heads 0+1 as concurrent M=64 column-tiles, head 2 split over
      even/odd kv tiles, PE-transpose [65,128] blocks back to [128,65],
      multiply by reciprocal denominator, DMA out.

Bias handling: bq optionally added in-kernel; bk cancels exactly in softmax
(constant along kv); bv added host-side (softmax rows sum to 1).
"""

import sys

sys.path.insert(0, "/opt/trn_rl_repo")

import numpy as np

from concourse import bacc, mybir, tile
from concourse.bass_utils import run_bass_kernel_spmd

F32 = mybir.dt.float32
BF16 = mybir.dt.bfloat16
EXP = mybir.ActivationFunctionType.Exp

B, S, H, NH, HD = 2, 2048, 768, 12, 64
NC = 8  # cores
HPC = 3  # heads per core
DL = HPC * HD  # 192 local columns
NT = S // 128  # 16 seq tiles
KT = H // 128  # 6 contraction tiles
QC = 512  # query chunk
NQC = S // QC  # 4
MJ = 3 * DL  # 576 joint QKV output columns
GROUPS = [(0, 2), (2, 2), (4, 2), (6, 2), (8, 2), (10, 2), (12, 2), (14, 2)]  # scores act groups

_CACHE = {}


def _build(use_qbias: bool):
    nc = bacc.Bacc("TRN2", target_bir_lowering=False, debug=False)
    hs_d = nc.dram_tensor("hs", [S, H], F32, kind="ExternalInput").ap()
    wf_d = nc.dram_tensor("wf", [H, MJ], F32, kind="ExternalInput").ap()
    eye_d = nc.dram_tensor("eye", [128, 128], F32, kind="ExternalInput").ap()
    out_d = nc.dram_tensor("out", [S, DL], F32, kind="ExternalOutput").ap()
    if use_qbias:
        bq_d = nc.dram_tensor("bq", [DL], F32, kind="ExternalInput").ap()

    ts = tile.bass.ts

    with tile.TileContext(nc) as tc:
        with tc.tile_pool(name="const", bufs=1) as cpool, \
             tc.tile_pool(name="qkv_sb", bufs=1) as qkv_pool, \
             tc.tile_pool(name="hsT_p", bufs=1) as hsT_pool:
            eye_f = cpool.tile([128, 128], F32)
            nc.sync.dma_start(eye_f[:], eye_d[:])
            eye_b = cpool.tile([128, 128], BF16)
            nc.vector.tensor_copy(eye_b[:], eye_f[:])
            ones_c = cpool.tile([128, 1], BF16)
            nc.vector.memset(ones_c[:], 1.0)
            if use_qbias:
                bq_sb = cpool.tile([128, 2, 1], F32)
                nc.sync.dma_start(bq_sb[0:128, 0, :], bq_d[0:128].rearrange("(p o) -> p o", o=1))
                nc.sync.dma_start(bq_sb[0:64, 1, :], bq_d[128:192].rearrange("(p o) -> p o", o=1))

            hsT = hsT_pool.tile([128, KT, S], BF16)
            # Q^T/K^T layouts: *_ab holds head0 on partitions 0-63, head1 on
            # 64-127; *_c2 holds head2 replicated on both halves.
            qt_ab = qkv_pool.tile([128, S], BF16)
            qt_c2 = qkv_pool.tile([128, S], BF16)
            kt_ab = qkv_pool.tile([128, S], BF16)
            kt_c2 = qkv_pool.tile([128, S], BF16)
            v1 = qkv_pool.tile([128, HPC, NT, HD], BF16)
            w_bb = qkv_pool.tile([128, KT, MJ], BF16)

            # ---- Phases A-C1 ----
            with tc.tile_pool(name="hs_p", bufs=1) as hs_pool, \
                 tc.tile_pool(name="w_p", bufs=1) as w_pool, \
                 tc.tile_pool(name="vt_p", bufs=1) as vt_pool, \
                 tc.tile_pool(name="early_ps", bufs=4, space="PSUM") as eps:
                hs_nat = hs_pool.tile([128, NT, H], F32)
                for t in range(NT):
                    nc.sync.dma_start(hs_nat[:, t, :], hs_d[ts(t, 128), :])
                w_sb = w_pool.tile([128, KT, MJ], F32)
                for k in range(KT):
                    nc.sync.dma_start(w_sb[:, k, :], wf_d[ts(k, 128), :])
                nc.vector.tensor_copy(w_bb[:], w_sb[:])

                # B: hs^T via PE transposes (f32 in, bf16 out via the copy)
                for t in range(NT):
                    for kg in range(2):
                        tp = eps.tile([128, 3, 128], F32, tag="bt", name=f"tpb{t}_{kg}")
                        for dk in range(3):
                            k = kg * 3 + dk
                            nc.tensor.transpose(
                                tp[:, dk, :], hs_nat[:, t, ts(k, 128)], eye_f[:]
                            )
                        nc.vector.tensor_copy(
                            hsT[:, kg * 3 : kg * 3 + 3, ts(t, 128)], tp[:]
                        )

                # C1: K,V projections (wf cols 192..576), n-chunk major
                vt_ab = vt_pool.tile([128, S], BF16)  # V^T heads 0|1
                vt_c = vt_pool.tile([64, S], BF16)  # V^T head 2
                for n in range(NQC):
                    for m in range(3):
                        m0 = 192 + m * 128
                        ps = eps.tile([128, QC], F32, tag="qk", name=f"kvps{n}_{m}")
                        for k in range(KT):
                            nc.tensor.matmul(
                                ps[:, :],
                                w_bb[:, k, m0 : m0 + 128],
                                hsT[:, k, ts(n, QC)],
                                start=(k == 0),
                                stop=(k == KT - 1),
                            )
                        if m == 0:  # K heads 0,1
                            nc.vector.tensor_copy(kt_ab[:, ts(n, QC)], ps[:])
                        elif m == 1:  # K head2 | V head0
                            nc.vector.tensor_copy(kt_c2[0:64, ts(n, QC)], ps[0:64, :])
                            nc.vector.tensor_copy(kt_c2[64:128, ts(n, QC)], ps[0:64, :])
                            nc.vector.tensor_copy(vt_ab[0:64, ts(n, QC)], ps[64:128, :])
                        else:  # V head1 | V head2
                            nc.vector.tensor_copy(vt_ab[64:128, ts(n, QC)], ps[0:64, :])
                            nc.vector.tensor_copy(vt_c[:, ts(n, QC)], ps[64:128, :])

                # C1b: V^T -> V natural
                for t in range(NT):
                    tpv = eps.tile([128, 128], BF16, tag="bt", name=f"tpv{t}")
                    nc.tensor.transpose(tpv[:], vt_ab[:, ts(t, 128)], eye_b[:])
                    nc.vector.tensor_copy(v1[:, 0, t, :], tpv[:, 0:64])
                    nc.vector.tensor_copy(v1[:, 1, t, :], tpv[:, 64:128])
                    tpc = eps.tile([128, 64], BF16, tag="bt", name=f"tpc{t}")
                    nc.tensor.transpose(tpc[:], vt_c[:, ts(t, 128)], eye_b[0:64, 0:64])
                    nc.vector.tensor_copy(v1[:, 2, t, :], tpc[:])

            # ---- Phase D: per q-chunk Q projection + attention ----
            with tc.tile_pool(name="et_p", bufs=5) as et_pool, \
                 tc.tile_pool(name="sm_p", bufs=2) as sm_pool, \
                 tc.tile_pool(name="cs_p", bufs=6) as cs_pool, \
                 tc.tile_pool(name="rd_p", bufs=8) as rd_pool, \
                 tc.tile_pool(name="sc_ps", bufs=3, space="PSUM") as sps, \
                 tc.tile_pool(name="cx_ps", bufs=2, space="PSUM") as cps_pool:
                for qc in range(NQC):
                    # Q^T projection for this q-chunk (wf cols 0..192)
                    for m in range(2):
                        mw = 128 if m == 0 else 64
                        qp = sps.tile([128, QC], F32, tag="sc", name=f"qps{qc}_{m}")
                        for k in range(KT):
                            nc.tensor.matmul(
                                qp[0:mw, :],
                                w_bb[:, k, m * 128 : m * 128 + mw],
                                hsT[:, k, ts(qc, QC)],
                                start=(k == 0),
                                stop=(k == KT - 1),
                            )
                        if m == 0:
                            if use_qbias:
                                nc.vector.tensor_scalar_add(
                                    qt_ab[:, ts(qc, QC)], qp[:], bq_sb[0:128, 0, :])
                            else:
                                nc.vector.tensor_copy(qt_ab[:, ts(qc, QC)], qp[:])
                        else:
                            for half in range(2):
                                d = qt_c2[half * 64 : half * 64 + 64, ts(qc, QC)]
                                if use_qbias:
                                    nc.vector.tensor_scalar_add(d, qp[0:64, :], bq_sb[0:64, 1, :])
                                else:
                                    nc.vector.tensor_copy(d, qp[0:64, :])

                    # scores + exp, heads 0/1 interleaved then head 2
                    et = [
                        et_pool.tile([128, NT, QC], BF16, tag="et", name=f"et{qc}_{h}")
                        for h in range(HPC)
                    ]
                    for g0, gl in GROUPS:
                        sA = sps.tile([128, 2, QC], F32, tag="sc", name=f"sA{qc}_{g0}")
                        sB = sps.tile([128, 2, QC], F32, tag="sc", name=f"sB{qc}_{g0}")
                        for i in range(gl):
                            t = g0 + i
                            nc.tensor.matmul(
                                sA[:, i, :],
                                kt_ab[0:64, ts(t, 128)],
                                qt_ab[0:64, ts(qc, QC)],
                                start=True, stop=True,
                            )
                            nc.tensor.matmul(
                                sB[:, i, :],
                                kt_ab[64:128, ts(t, 128)],
                                qt_ab[64:128, ts(qc, QC)],
                                start=True, stop=True,
                            )
                        nc.scalar.activation(
                            et[0][:, g0 : g0 + gl, :], sA[:, 0:gl, :], EXP, scale=0.125)
                        nc.scalar.activation(
                            et[1][:, g0 : g0 + gl, :], sB[:, 0:gl, :], EXP, scale=0.125)
                    for g0, gl in GROUPS:
                        sC = sps.tile([128, 2, QC], F32, tag="sc", name=f"sC{qc}_{g0}")
                        for i in range(gl):
                            t = g0 + i
                            hh = t % 2
                            nc.tensor.matmul(
                                sC[:, i, :],
                                kt_c2[hh * 64 : hh * 64 + 64, ts(t, 128)],
                                qt_c2[hh * 64 : hh * 64 + 64, ts(qc, QC)],
                                start=True, stop=True,
                            )
                        nc.scalar.activation(
                            et[2][:, g0 : g0 + gl, :], sC[:, 0:gl, :], EXP, scale=0.125)

                    cs = [
                        cs_pool.tile([HD + 1, QC], BF16, tag="cs", name=f"cs{qc}_{h}")
                        for h in range(HPC)
                    ]
                    # R3: denominators, three concurrent M=1 column-tiles
                    dps = cps_pool.tile([128, QC], F32, tag="cx", name=f"dps{qc}")
                    for t in range(NT):
                        for h in range(HPC):
                            nc.tensor.matmul(
                                dps[32 * h : 32 * h + 1, :],
                                ones_c[:],
                                et[h][:, t, :],
                                start=(t == 0), stop=(t == NT - 1),
                                tile_position=(0, 32 * h),
                            )
                    for h in range(HPC):
                        nc.vector.tensor_copy(
                            cs[h][HD : HD + 1, :], dps[32 * h : 32 * h + 1, :])

                    # R1: ctx heads 0+1 as concurrent column-tiles
                    c01 = cps_pool.tile([128, QC], F32, tag="cx", name=f"c01{qc}")
                    for t in range(NT):
                        nc.tensor.matmul(
                            c01[0:64, :], v1[:, 0, t, :], et[0][:, t, :],
                            start=(t == 0), stop=(t == NT - 1),
                            tile_position=(0, 0),
                        )
                        nc.tensor.matmul(
                            c01[64:128, :], v1[:, 1, t, :], et[1][:, t, :],
                            start=(t == 0), stop=(t == NT - 1),
                            tile_position=(0, 64),
                        )
                    nc.vector.tensor_copy(cs[0][0:HD, :], c01[0:64, :])
                    nc.vector.tensor_copy(cs[1][0:HD, :], c01[64:128, :])

                    # R2: ctx head 2 split over even/odd kv tiles
                    c2 = cps_pool.tile([128, QC], F32, tag="cx", name=f"c2{qc}")
                    for tt in range(NT // 2):
                        nc.tensor.matmul(
                            c2[0:64, :], v1[:, 2, 2 * tt, :], et[2][:, 2 * tt, :],
                            start=(tt == 0), stop=(tt == NT // 2 - 1),
                            tile_position=(0, 0),
                        )
                        nc.tensor.matmul(
                            c2[64:128, :], v1[:, 2, 2 * tt + 1, :], et[2][:, 2 * tt + 1, :],
                            start=(tt == 0), stop=(tt == NT // 2 - 1),
                            tile_position=(0, 64),
                        )
                    c2t = cs_pool.tile([64, QC], BF16, tag="c2t", name=f"c2t{qc}")
                    nc.vector.tensor_copy(c2t[:], c2[64:128, :])
                    nc.vector.tensor_add(cs[2][0:HD, :], c2[0:64, :], c2t[:])

                    # transpose [65,128] blocks -> [128,66], normalize, stage out
                    osb = sm_pool.tile([128, NQC, DL], F32, tag="osb", name=f"osb{qc}")
                    tp12 = cps_pool.tile([128, HPC, 4, HD + 2], BF16, tag="cx", name=f"tp12{qc}")
                    for h in range(HPC):
                        for j in range(4):
                            nc.tensor.transpose(
                                tp12[:, h, j, 0 : HD + 1], cs[h][:, ts(j, 128)],
                                eye_b[0 : HD + 1, 0 : HD + 1],
                            )
                    for h in range(HPC):
                        for j in range(4):
                            rd = rd_pool.tile([128, 1], F32, tag="rd", name=f"rd{qc}_{h}_{j}")
                            nc.vector.reciprocal(rd[:], tp12[:, h, j, HD : HD + 1])
                            nc.vector.tensor_scalar_mul(
                                osb[:, j, h * HD : (h + 1) * HD],
                                tp12[:, h, j, 0:HD],
                                rd[:],
                            )
                    nc.sync.dma_start(
                        out_d[ts(qc, QC), :].rearrange("(j p) d -> p j d", p=128),
                        osb[:],
                    )

    nc.compile()
    return nc


def _get(use_qbias: bool):
    key = use_qbias
    if key not in _CACHE:
        _CACHE[key] = _build(use_qbias)
    return _CACHE[key]


def _make_in_maps(hidden_states, Wq, bq, Wk, Wv, use_qbias):
    eye = np.eye(128, dtype=np.float32)
    in_maps = []
    for i in range(NC):
        b, g = divmod(i, NC // B)
        c0 = g * DL
        m = {
            "hs": np.ascontiguousarray(hidden_states[b], dtype=np.float32),
            "wf": np.ascontiguousarray(
                np.concatenate(
                    [Wq[:, c0 : c0 + DL], Wk[:, c0 : c0 + DL], Wv[:, c0 : c0 + DL]],
                    axis=1,
                ),
                dtype=np.float32,
            ),
            "eye": eye,
        }
        if use_qbias:
            m["bq"] = np.ascontiguousarray(bq[c0 : c0 + DL], dtype=np.float32)
        in_maps.append(m)
    return in_maps


def _run(inputs, trace=False):
    hidden_states = np.asarray(inputs["hidden_states"], dtype=np.float32)
    Wq = np.asarray(inputs["Wq"], dtype=np.float32)
    Wk = np.asarray(inputs["Wk"], dtype=np.float32)
    Wv = np.asarray(inputs["Wv"], dtype=np.float32)
    bq = np.asarray(inputs["bq"], dtype=np.float32)
    bv = np.asarray(inputs["bv"], dtype=np.float32)
    # bk is intentionally unused: softmax over the kv axis cancels any
    # per-query constant, and q_i . bk is constant along kv.
    assert hidden_states.shape == (B, S, H)
    use_qbias = bool(np.any(bq))
    nc = _get(use_qbias)
    in_maps = _make_in_maps(hidden_states, Wq, bq, Wk, Wv, use_qbias)
    res = run_bass_kernel_spmd(nc, in_maps, core_ids=list(range(NC)), trace=trace)
    out = np.empty((B, S, H), dtype=np.float32)
    for i in range(NC):
        b, g = divmod(i, NC // B)
        c0 = g * DL
        out[b, :, c0 : c0 + DL] = res.results[i]["out"] + bv[c0 : c0 + DL]
    return out, res


def kernel(**inputs) -> np.ndarray:
    out, _ = _run(inputs, trace=False)
    return out
